# revision 3
# baseline (speedup 1.0000x reference)
"""Geometric-product 3D conv (Cl(3,0) GA conv) on 8 Trainium2 NeuronCores.

Problem: x[B=8, Cin=8, I=8, 48,48,48], W[3,3,3, Cin=8, Cout=8, J=8], b[8,8],
G[8,8,8] ->  out[B=8, Cout=8, K=8, 46,46,46]   (VALID 3d conv after folding
the geometric-product table G into the weights).

Strategy:
  * Fold G into W on host -> dense conv kernel Kfold[o'=64, c'=64, 3,3,3].
  * Data parallel: one batch element per NeuronCore (8 cores).
  * Conv as matmul with d-parity packing: SBUF x layout has partitions
    p = c'*2 + (d mod 2), so each "d-pair" tile [128, 48*48] holds two
    adjacent depth slices.  Output likewise packs (o', d mod 2) on its
    128 partitions.  One output d-pair needs x d-rows 2Q..2Q+3 = x-pair
    tiles Q and Q+1, giving 2 accumulation steps x 9 (l,v) kernel offsets
    = 18 matmuls of [K=128, M=128] x [128, N] per PSUM tile (75% PE
    utilization vs 37.5% for naive K=64).
  * bf16 operands (fp32 PSUM accumulate): enables FWL fast weight load
    and halves DMA traffic; rel err ~2.5e-3 (vs 1.6e-4 fp32r), inside
    the 2e-2 gate.
  * The (h,w) plane is computed on the full 48-wide grid (stride
    alignment with the input), chunked linearly in f = h*48+w into
    four 512-col + one 160-col PSUM banks; output rows stay 48 wide in
    DRAM and the 2 garbage w-columns are cropped on host.
  * One output DMA per d-pair ([128 x 2208] bf16, 4416B per-partition
    descriptors) instead of per-h-chunk 184B-descriptor stores.
"""

import sys

import numpy as np

sys.path.insert(0, "/opt/trn_rl_repo")

_PROGRAM = None

# f-chunks of the 46x48 output plane (f = h*48 + w); N<=512 fp32 PSUM bank
_CHUNKS = [(0, 512), (512, 512), (1024, 512), (1536, 512), (2048, 160)]


def _build_program(chunks=None, xp_bufs=6, ps_mode="tagged", ot_bufs=3, repeat=0,
                   mm_order="weight", dedup_ldw=True):
    import contextlib

    import concourse.bacc as bacc
    import concourse.mybir as mybir
    from concourse import tile

    chunks = chunks or _CHUNKS
    f32 = mybir.dt.float32
    bf16 = mybir.dt.bfloat16

    nc = bacc.Bacc(None, target_bir_lowering=False)
    # 49th d-row is host-side zero padding so the tail-pad DMA of the last
    # d-pair stays in bounds.
    x_in = nc.declare_dram_parameter("x", [64, 49, 2304], bf16, isOutput=False)
    wt_in = nc.declare_dram_parameter("wt", [128, 18 * 128], bf16, isOutput=False)
    b_in = nc.declare_dram_parameter("bias", [128, 1], f32, isOutput=False)
    # [o', d%2, d//2, f=h*48+w]: leading (o', dp) merge to the 128 SBUF
    # partitions in one DMA AP dim (stride ratio is exactly 2); host
    # untangles d = 2q+dp and crops w 48->46.
    out_ext = nc.declare_dram_parameter("out", [64, 2, 23, 2208], bf16, isOutput=True)

    with tile.TileContext(nc) as tc:
        with (
            tc.tile_pool(name="wt", bufs=1) as wtp,
            tc.tile_pool(name="xp", bufs=xp_bufs) as xpp,
            tc.tile_pool(
                name="ps", bufs=7 if ps_mode == "shared" else 1, space="PSUM"
            ) as psp,
            tc.tile_pool(name="ot", bufs=ot_bufs) as otp,
            tc.tile_pool(name="bias", bufs=1) as bp,
        ):
            # repeat>0 wraps the whole body in a HW loop (benchmarking only)
            rep_ctx = tc.For_i(0, repeat, 1) if repeat else contextlib.nullcontext()
            with rep_ctx:
                _emit_body(nc, tc, chunks, ps_mode, mm_order, wtp, xpp, psp, otp,
                           bp, x_in, wt_in, b_in, out_ext, f32, bf16, mybir)
    nc.finalize()
    if dedup_ldw:
        _dedup_ldweights(nc, mybir)
    return nc


def _dedup_ldweights(nc, mybir):
    """Remove InstLdweights that reload the already-loaded stationary weights
    (emitted when consecutive matmuls share the same lhsT).  Their semaphore
    waits are transferred to the next PE instruction."""

    def sig(x):
        ap = x.ins[0]
        return (ap.offset, str(ap.memref), str(ap.ap), str(ap.dtype),
                str(getattr(x, "tile_position", None)))

    removed = 0
    for blk in nc.m.functions[0].blocks:
        ins = blk.instructions
        last = None
        pending = []
        kill = []
        for i in range(len(ins)):
            x = ins[i]
            nm = type(x).__name__
            if str(x.engine) != "EngineType.PE":
                continue
            if nm == "InstLdweights":
                s = sig(x)
                if s == last:
                    si = x.sync_info
                    if si is not None and len(si.on_wait):
                        pending.extend(si.on_wait)
                        assert not len(si.on_update)
                    kill.append(i)
                    continue
                last = s
            if pending:
                si = x.sync_info
                if si is None:
                    x.sync_info = mybir.SyncInfo(on_wait=pending, on_update=[])
                else:
                    si.on_wait = list(si.on_wait) + pending
                pending = []
        assert not pending
        for i in reversed(kill):
            del ins[i]
        removed += len(kill)
    return removed


def _emit_body(nc, tc, chunks, ps_mode, mm_order, wtp, xpp, psp, otp, bp,
               x_in, wt_in, b_in, out_ext, f32, bf16, mybir):
    wt = wtp.tile([128, 18 * 128], bf16)
    nc.sync.dma_start(out=wt[:], in_=wt_in[:])
    bias = bp.tile([128, 1], f32)
    nc.sync.dma_start(out=bias[:], in_=b_in[:])

    xp = {}

    def load_xpair(j):
        # [128, 2304] payload + 64-elem tail pad: shifted matmul reads
        # for the last f-chunk run up to 2 elements past the payload.
        # The pad holds the start of the following d-row (values land
        # only in cropped garbage columns).
        t = xpp.tile([128, 2368], bf16, tag="xp")
        nc.sync.dma_start(out=t[:, 0:2304], in_=x_in[:, 2 * j : 2 * j + 2, :])
        nc.sync.dma_start(
            out=t[:, 2304:2368], in_=x_in[:, 2 * j + 1 : 2 * j + 3, 0:64]
        )
        xp[j] = t

    load_xpair(0)
    load_xpair(1)

    ident = mybir.ActivationFunctionType.Identity

    for Q in range(23):
        if Q + 2 <= 23:
            load_xpair(Q + 2)
        ot = otp.tile([128, 2208], bf16, tag="ot")
        pss = [
            psp.tile([128, N], f32, tag=f"ps{ci}", name=f"ps{ci}")
            for ci, (f0, N) in enumerate(chunks)
        ]

        def mm(ci, k, s, l, v):
            f0, N = chunks[ci]
            off = f0 + l * 48 + v
            nc.tensor.matmul(
                pss[ci][:],
                lhsT=wt[:, 128 * k : 128 * (k + 1)],
                rhs=xp[Q + s][:, off : off + N],
                start=(k == 0),
                stop=(k == 17),
            )

        if mm_order == "weight":
            # stationary-weight reuse: LDW once per (k), 5 matmuls
            for k in range(18):
                s, l, v = k // 9, (k % 9) // 3, k % 3
                for ci in range(len(chunks)):
                    mm(ci, k, s, l, v)
        else:
            for ci in range(len(chunks)):
                for k in range(18):
                    s, l, v = k // 9, (k % 9) // 3, k % 3
                    mm(ci, k, s, l, v)

        # drain PSUM -> ot with bias add, split across DVE and ACT so the
        # banks free up ~2x faster for the next Q's matmuls
        for ci, (f0, N) in enumerate(chunks):
            if ci < 3:
                nc.vector.tensor_scalar_add(
                    out=ot[:, f0 : f0 + N], in0=pss[ci][:], scalar1=bias[:]
                )
            else:
                nc.scalar.activation(
                    out=ot[:, f0 : f0 + N], in_=pss[ci][:], func=ident,
                    bias=bias[:], scale=1.0,
                )
        nc.sync.dma_start(out=out_ext[:, :, Q, :], in_=ot[:])


def _get_program():
    global _PROGRAM
    if _PROGRAM is None:
        _PROGRAM = _build_program()
    return _PROGRAM


def _prepare_host_inputs(x, W, b, G):
    import ml_dtypes

    bf16 = ml_dtypes.bfloat16
    B = x.shape[0]
    # Fold GA product table into the conv kernel:
    # out[b,o,k,d,h,w] = sum G[i,j,k] x[b,c,i,...] W[m,l,v,c,o,j]
    Wt = np.einsum("ijk,mlvcoj->okcimlv", G, W).astype(np.float32)
    Kfold = np.ascontiguousarray(Wt.reshape(64, 64, 3, 3, 3))  # [o', c', m, l, v]

    # 18 stationary matrices: lhsT[k_in = c'*2+dpi, p_out = o'*2+dpo]
    WBIG = np.zeros((128, 18, 128), np.float32)
    L = np.zeros((64, 2, 64, 2), np.float32)  # [c', dpi, o', dpo]
    for s in (0, 1):
        for l in range(3):
            for v in range(3):
                k = s * 9 + l * 3 + v
                L[:] = 0.0
                for dpi in (0, 1):
                    for dpo in (0, 1):
                        m = 2 * s + dpi - dpo
                        if 0 <= m <= 2:
                            L[:, dpi, :, dpo] = Kfold[:, :, m, l, v].T
                WBIG[:, k, :] = L.reshape(128, 128)
    wt_arr = np.ascontiguousarray(WBIG.reshape(128, 18 * 128)).astype(bf16)

    bias_arr = np.repeat(b.reshape(64).astype(np.float32), 2).reshape(128, 1)
    bias_arr = np.ascontiguousarray(bias_arr)

    zrow = np.zeros((64, 1, 2304), np.float32)
    xs = [
        np.ascontiguousarray(
            np.concatenate([x[i].reshape(64, 48, 2304), zrow], axis=1)
        ).astype(bf16)
        for i in range(B)
    ]
    return xs, wt_arr, bias_arr


def kernel(**inputs):
    from concourse.bass_utils import run_bass_kernel_spmd

    x = np.asarray(inputs["x"], np.float32)
    W = np.asarray(inputs["W"], np.float32)
    b = np.asarray(inputs["b"], np.float32)
    G = np.asarray(inputs["G"], np.float32)

    xs, wt_arr, bias_arr = _prepare_host_inputs(x, W, b, G)
    nc = _get_program()
    in_maps = [{"x": xs[i], "wt": wt_arr, "bias": bias_arr} for i in range(8)]
    res = run_bass_kernel_spmd(nc, in_maps, list(range(8)))
    out = np.stack([_unpack_out(res.results[i]["out"]) for i in range(8)], axis=0)
    return out.reshape(8, 8, 8, 46, 46, 46)


def _unpack_out(arr):
    # [o', dp, q, f=h*48+w] -> [o', d=2q+dp, h, w] cropped to w<46, fp32
    a = np.asarray(arr, np.float32).reshape(64, 2, 23, 46, 48)[:, :, :, :, 0:46]
    return np.ascontiguousarray(a.transpose(0, 2, 1, 3, 4)).reshape(64, 46, 46, 46)


# revision 9
# speedup vs baseline: 2.4079x; 2.4079x over previous
"""Geometric-product 3D conv (Cl(3,0) GA conv) on 8 Trainium2 NeuronCores.

Problem: x[B=8, Cin=8, I=8, 48,48,48], W[3,3,3, Cin=8, Cout=8, J=8], b[8,8],
G[8,8,8] ->  out[B=8, Cout=8, K=8, 46,46,46]   (VALID 3d conv after folding
the geometric-product table G into the weights).

Strategy:
  * Fold G into W on host -> dense conv kernel Kfold[o'=64, c'=64, 3,3,3].
  * Data parallel: one batch element per NeuronCore (8 cores).
  * Conv as matmul with d-parity packing: SBUF x layout has partitions
    p = c'*2 + (d mod 2), so each "d-pair" tile [128, 48*48] holds two
    adjacent depth slices.  Output likewise packs (o', d mod 2) on its
    128 partitions.  One output d-pair needs x d-rows 2Q..2Q+3 = x-pair
    tiles Q and Q+1, giving 2 accumulation steps x 9 (l,v) kernel offsets
    = 18 matmuls of [K=128, M=128] x [128, N] per PSUM tile (75% PE
    utilization vs 37.5% for naive K=64).
  * bf16 operands (fp32 PSUM accumulate): enables FWL fast weight load
    and halves DMA traffic; rel err ~2.5e-3 (vs 1.6e-4 fp32r), inside
    the 2e-2 gate.
  * The (h,w) plane is computed on the full 48-wide grid (stride
    alignment with the input), chunked linearly in f = h*48+w into
    four 512-col + one 160-col PSUM banks; output rows stay 48 wide in
    DRAM and the 2 garbage w-columns are cropped on host.
  * One output DMA per d-pair ([128 x 2208] bf16, 4416B per-partition
    descriptors) instead of per-h-chunk 184B-descriptor stores.
"""

import sys

import numpy as np

sys.path.insert(0, "/opt/trn_rl_repo")

_PROGRAM = None

# f-chunks of the 46x48 output plane (f = h*48 + w); N<=512 fp32 PSUM bank
_CHUNKS = [(0, 512), (512, 512), (1024, 512), (1536, 512), (2048, 158)]


def _build_program(chunks=None, xp_bufs=6, ps_mode="tagged", ot_bufs=3, repeat=0,
                   mm_order="chunk", dedup_ldw=True, ablate="none"):
    import contextlib

    import concourse.bacc as bacc
    import concourse.mybir as mybir
    from concourse import tile

    chunks = chunks or _CHUNKS
    f32 = mybir.dt.float32
    bf16 = mybir.dt.bfloat16

    nc = bacc.Bacc(None, target_bir_lowering=False)
    # 49th d-row is host-side zero padding so the tail-pad DMA of the last
    # d-pair stays in bounds.
    x_in = nc.declare_dram_parameter("x", [64, 49, 2304], bf16, isOutput=False)
    wt_in = nc.declare_dram_parameter("wt", [128, 18 * 128], bf16, isOutput=False)
    b_in = nc.declare_dram_parameter("bias", [128, 1], f32, isOutput=False)
    # [o', d%2, d//2, f=h*48+w]: leading (o', dp) merge to the 128 SBUF
    # partitions in one DMA AP dim (stride ratio is exactly 2); host
    # untangles d = 2q+dp and crops w 48->46.
    out_ext = nc.declare_dram_parameter("out", [64, 2, 23, 2208], bf16, isOutput=True)

    with tile.TileContext(nc) as tc:
        with (
            tc.tile_pool(name="wt", bufs=1) as wtp,
            tc.tile_pool(name="xp", bufs=xp_bufs) as xpp,
            tc.tile_pool(
                name="ps", bufs=7 if ps_mode == "shared" else 1, space="PSUM"
            ) as psp,
            tc.tile_pool(name="ot", bufs=ot_bufs) as otp,
            tc.tile_pool(name="bias", bufs=1) as bp,
        ):
            # repeat>0 wraps the whole body in a HW loop (benchmarking only)
            rep_ctx = tc.For_i(0, repeat, 1) if repeat else contextlib.nullcontext()
            with rep_ctx:
                _emit_body(nc, tc, chunks, ps_mode, mm_order, wtp, xpp, psp, otp,
                           bp, x_in, wt_in, b_in, out_ext, f32, bf16, mybir,
                           ablate)
    nc.finalize()
    if dedup_ldw:
        _dedup_ldweights(nc, mybir)
    return nc


def _dedup_ldweights(nc, mybir):
    """Remove InstLdweights that reload the already-loaded stationary weights
    (emitted when consecutive matmuls share the same lhsT).  Their semaphore
    waits are transferred to the next PE instruction."""

    def sig(x):
        ap = x.ins[0]
        return (ap.offset, str(ap.memref), str(ap.ap), str(ap.dtype),
                str(getattr(x, "tile_position", None)))

    removed = 0
    for blk in nc.m.functions[0].blocks:
        ins = blk.instructions
        last = None
        pending = []
        kill = []
        for i in range(len(ins)):
            x = ins[i]
            nm = type(x).__name__
            if str(x.engine) != "EngineType.PE":
                continue
            if nm == "InstLdweights":
                s = sig(x)
                if s == last:
                    si = x.sync_info
                    if si is not None and len(si.on_wait):
                        pending.extend(si.on_wait)
                        assert not len(si.on_update)
                    kill.append(i)
                    continue
                last = s
            if pending:
                si = x.sync_info
                if si is None:
                    x.sync_info = mybir.SyncInfo(on_wait=pending, on_update=[])
                else:
                    si.on_wait = list(si.on_wait) + pending
                pending = []
        assert not pending
        for i in reversed(kill):
            del ins[i]
        removed += len(kill)
    return removed


def _emit_body(nc, tc, chunks, ps_mode, mm_order, wtp, xpp, psp, otp, bp,
               x_in, wt_in, b_in, out_ext, f32, bf16, mybir, ablate="none"):
    wt = wtp.tile([128, 18 * 128], bf16)
    nc.sync.dma_start(out=wt[:], in_=wt_in[:])
    bias = bp.tile([128, 1], f32)
    nc.sync.dma_start(out=bias[:], in_=b_in[:])

    xp = {}

    def load_xpair(j):
        # single [128, 2304] DMA per d-pair; the last f-chunk is 158 wide so
        # every shifted matmul read stays inside the payload (no tail pad)
        t = xpp.tile([128, 2304], bf16, tag="xp")
        nc.sync.dma_start(out=t[:], in_=x_in[:, 2 * j : 2 * j + 2, :])
        xp[j] = t

    load_xpair(0)
    if ablate not in ("no_xload", "mm_only"):
        load_xpair(1)

    ident = mybir.ActivationFunctionType.Identity

    for Q in range(23):
        if Q + 2 <= 23 and ablate not in ("no_xload", "mm_only"):
            load_xpair(Q + 2)
        ot = otp.tile([128, 2208], bf16, tag="ot")
        # chunks 0-2 double-buffer across Q parity (3*2+2 = 8 PSUM banks):
        # next Q's matmuls never wait on this Q's drains
        pss = [
            psp.tile(
                [128, N], f32,
                tag=f"ps{ci}q{Q % 2}" if ci < 3 else f"ps{ci}",
                name=f"ps{ci}",
            )
            for ci, (f0, N) in enumerate(chunks)
        ]

        def mm(ci, k, s, l, v):
            f0, N = chunks[ci]
            off = f0 + l * 48 + v
            rt = xp[0] if ablate in ("no_xload", "mm_only") else xp[Q + s]
            nc.tensor.matmul(
                pss[ci][:],
                lhsT=wt[:, 128 * k : 128 * (k + 1)],
                rhs=rt[:, off : off + N],
                start=(k == 0),
                stop=(k == 17),
            )

        if mm_order == "weight":
            # stationary-weight reuse: LDW once per (k), 5 matmuls
            for k in range(18):
                s, l, v = k // 9, (k % 9) // 3, k % 3
                for ci in range(len(chunks)):
                    mm(ci, k, s, l, v)
        else:
            for ci in range(len(chunks)):
                for k in range(18):
                    s, l, v = k // 9, (k % 9) // 3, k % 3
                    mm(ci, k, s, l, v)

        # drain PSUM -> ot with bias add, split across DVE and ACT so the
        # banks free up ~2x faster for the next Q's matmuls
        if ablate in ("min_io", "mm_only"):
            # keep only a minimal drain+store of the last (160-col) bank so
            # DCE cannot remove the matmuls (benchmark ablation only)
            f0, N = chunks[-1]
            nc.vector.tensor_scalar_add(
                out=ot[:, f0 : f0 + N], in0=pss[-1][:], scalar1=bias[:]
            )
            nc.scalar.dma_start(
                out=out_ext[:, :, Q, f0 : f0 + N], in_=ot[:, f0 : f0 + N]
            )
            continue
        for ci, (f0, N) in enumerate(chunks):
            if ci < 3:
                nc.vector.tensor_scalar_add(
                    out=ot[:, f0 : f0 + N], in0=pss[ci][:], scalar1=bias[:]
                )
            else:
                nc.scalar.activation(
                    out=ot[:, f0 : f0 + N], in_=pss[ci][:], func=ident,
                    bias=bias[:], scale=1.0,
                )
        nc.scalar.dma_start(out=out_ext[:, :, Q, 0:2206], in_=ot[:, 0:2206])


def _get_program():
    global _PROGRAM
    if _PROGRAM is None:
        _PROGRAM = _build_program()
    return _PROGRAM


def _prepare_host_inputs(x, W, b, G):
    import ml_dtypes

    bf16 = ml_dtypes.bfloat16
    B = x.shape[0]
    # Fold GA product table into the conv kernel:
    # out[b,o,k,d,h,w] = sum G[i,j,k] x[b,c,i,...] W[m,l,v,c,o,j]
    Wt = np.einsum("ijk,mlvcoj->okcimlv", G, W).astype(np.float32)
    Kfold = np.ascontiguousarray(Wt.reshape(64, 64, 3, 3, 3))  # [o', c', m, l, v]

    # 18 stationary matrices: lhsT[k_in = c'*2+dpi, p_out = o'*2+dpo]
    WBIG = np.zeros((128, 18, 128), np.float32)
    L = np.zeros((64, 2, 64, 2), np.float32)  # [c', dpi, o', dpo]
    for s in (0, 1):
        for l in range(3):
            for v in range(3):
                k = s * 9 + l * 3 + v
                L[:] = 0.0
                for dpi in (0, 1):
                    for dpo in (0, 1):
                        m = 2 * s + dpi - dpo
                        if 0 <= m <= 2:
                            L[:, dpi, :, dpo] = Kfold[:, :, m, l, v].T
                WBIG[:, k, :] = L.reshape(128, 128)
    wt_arr = np.ascontiguousarray(WBIG.reshape(128, 18 * 128)).astype(bf16)

    bias_arr = np.repeat(b.reshape(64).astype(np.float32), 2).reshape(128, 1)
    bias_arr = np.ascontiguousarray(bias_arr)

    zrow = np.zeros((64, 1, 2304), np.float32)
    xs = [
        np.ascontiguousarray(
            np.concatenate([x[i].reshape(64, 48, 2304), zrow], axis=1)
        ).astype(bf16)
        for i in range(B)
    ]
    return xs, wt_arr, bias_arr


def kernel(**inputs):
    from concourse.bass_utils import run_bass_kernel_spmd

    x = np.asarray(inputs["x"], np.float32)
    W = np.asarray(inputs["W"], np.float32)
    b = np.asarray(inputs["b"], np.float32)
    G = np.asarray(inputs["G"], np.float32)

    xs, wt_arr, bias_arr = _prepare_host_inputs(x, W, b, G)
    nc = _get_program()
    in_maps = [{"x": xs[i], "wt": wt_arr, "bias": bias_arr} for i in range(8)]
    res = run_bass_kernel_spmd(nc, in_maps, list(range(8)))
    out = np.stack([_unpack_out(res.results[i]["out"]) for i in range(8)], axis=0)
    return out.reshape(8, 8, 8, 46, 46, 46)


def _unpack_out(arr):
    # [o', dp, q, f=h*48+w] -> [o', d=2q+dp, h, w] cropped to w<46, fp32
    a = np.asarray(arr, np.float32).reshape(64, 2, 23, 46, 48)[:, :, :, :, 0:46]
    return np.ascontiguousarray(a.transpose(0, 2, 1, 3, 4)).reshape(64, 46, 46, 46)


# revision 12
# speedup vs baseline: 3.3111x; 1.3751x over previous
"""Geometric-product 3D conv (Cl(3,0) GA conv) on 8 Trainium2 NeuronCores.

Problem: x[B=8, Cin=8, I=8, 48,48,48], W[3,3,3, Cin=8, Cout=8, J=8], b[8,8],
G[8,8,8] ->  out[B=8, Cout=8, K=8, 46,46,46]   (VALID 3d conv after folding
the geometric-product table G into the weights).

Strategy:
  * Fold G into W on host -> dense conv kernel Kfold[o'=64, c'=64, 3,3,3].
  * Data parallel: one batch element per NeuronCore (8 cores).
  * Conv as matmul with d-parity packing: SBUF x layout has partitions
    p = c'*2 + (d mod 2), so each "d-pair" tile [128, 48*48] holds two
    adjacent depth slices.  Output likewise packs (o', d mod 2) on its
    128 partitions.  One output d-pair needs x d-rows 2Q..2Q+3 = x-pair
    tiles Q and Q+1, giving 2 accumulation steps x 9 (l,v) kernel offsets
    = 18 matmuls of [K=128, M=128] x [128, N] per PSUM tile (75% PE
    utilization vs 37.5% for naive K=64).
  * bf16 operands (fp32 PSUM accumulate): pipelined/FWL weight loads
    and half the DMA traffic; rel err ~4e-3 (vs 1.6e-4 fp32r), inside
    the 2e-2 gate.
  * The (h,w) plane is computed on the full 48-wide grid (stride
    alignment with the input), chunked linearly in f = h*48+w into
    four 512-col + one 158-col PSUM banks (the 158 tail keeps every
    shifted read inside the 2304-col x payload -> no pad DMAs); output
    rows stay 48 wide in DRAM, garbage w-columns cropped on host.
    Chunks 0-2 double-buffer across Q parity (8 PSUM banks total) and
    drains split DVE/ACT so next-Q matmuls never wait on drains.
  * One output DMA per d-pair ([128 x 2206] bf16, 4.4KB per-partition
    descriptors) instead of per-h-chunk 184B-descriptor stores.
  * DMA queue decoupling (the big one, 1128us -> ~600us): stores issue
    on the scalar-engine HWDGE ring, loads on the sync ring, and a
    dummy 50th DMA per iteration keeps the tile framework's global
    round-robin DMAHW0-7 completion-sem lanes parity-stable so loads
    (even lanes) never share a lane with stores (odd lanes).  Without
    this, a load's ring-recycle wait chains to a store completion,
    which waits on drains -> matmuls -> effectively serializing DMA
    with compute (measured: each stream alone overlaps fine at ~498us,
    both together ballooned to 1128us).
"""

import sys

import numpy as np

sys.path.insert(0, "/opt/trn_rl_repo")

_PROGRAM = None

# f-chunks of the 46x48 output plane (f = h*48 + w); N<=512 fp32 PSUM bank
_CHUNKS = [(0, 512), (512, 512), (1024, 512), (1536, 512), (2048, 158)]


def _build_program(chunks=None, xp_bufs=6, ps_mode="tagged", ot_bufs=3, repeat=0,
                   mm_order="chunk", dedup_ldw=True, ablate="none"):
    import contextlib

    import concourse.bacc as bacc
    import concourse.mybir as mybir
    from concourse import tile

    chunks = chunks or _CHUNKS
    f32 = mybir.dt.float32
    bf16 = mybir.dt.bfloat16

    nc = bacc.Bacc(None, target_bir_lowering=False)
    # 49th d-row is host-side zero padding so the tail-pad DMA of the last
    # d-pair stays in bounds.
    x_in = nc.declare_dram_parameter("x", [64, 49, 2304], bf16, isOutput=False)
    wt_in = nc.declare_dram_parameter("wt", [128, 18 * 128], bf16, isOutput=False)
    b_in = nc.declare_dram_parameter("bias", [128, 1], f32, isOutput=False)
    # [o', d%2, d//2, f=h*48+w]: leading (o', dp) merge to the 128 SBUF
    # partitions in one DMA AP dim (stride ratio is exactly 2); host
    # untangles d = 2q+dp and crops w 48->46.
    out_ext = nc.declare_dram_parameter("out", [64, 2, 23, 2208], bf16, isOutput=True)

    with tile.TileContext(nc) as tc:
        with (
            tc.tile_pool(name="wt", bufs=1) as wtp,
            tc.tile_pool(name="xp", bufs=xp_bufs) as xpp,
            tc.tile_pool(
                name="ps", bufs=7 if ps_mode == "shared" else 1, space="PSUM"
            ) as psp,
            tc.tile_pool(name="ot", bufs=ot_bufs) as otp,
            tc.tile_pool(name="bias", bufs=1) as bp,
        ):
            # repeat>0 wraps the whole body in a HW loop (benchmarking only)
            rep_ctx = tc.For_i(0, repeat, 1) if repeat else contextlib.nullcontext()
            with rep_ctx:
                _emit_body(nc, tc, chunks, ps_mode, mm_order, wtp, xpp, psp, otp,
                           bp, x_in, wt_in, b_in, out_ext, f32, bf16, mybir,
                           ablate)
    nc.finalize()
    if dedup_ldw:
        _dedup_ldweights(nc, mybir)
    return nc


def _dedup_ldweights(nc, mybir):
    """Remove InstLdweights that reload the already-loaded stationary weights
    (emitted when consecutive matmuls share the same lhsT).  Their semaphore
    waits are transferred to the next PE instruction."""

    def sig(x):
        ap = x.ins[0]
        return (ap.offset, str(ap.memref), str(ap.ap), str(ap.dtype),
                str(getattr(x, "tile_position", None)))

    removed = 0
    for blk in nc.m.functions[0].blocks:
        ins = blk.instructions
        last = None
        pending = []
        kill = []
        for i in range(len(ins)):
            x = ins[i]
            nm = type(x).__name__
            if str(x.engine) != "EngineType.PE":
                continue
            if nm == "InstLdweights":
                s = sig(x)
                if s == last:
                    si = x.sync_info
                    if si is not None and len(si.on_wait):
                        pending.extend(si.on_wait)
                        assert not len(si.on_update)
                    kill.append(i)
                    continue
                last = s
            if pending:
                si = x.sync_info
                if si is None:
                    x.sync_info = mybir.SyncInfo(on_wait=pending, on_update=[])
                else:
                    si.on_wait = list(si.on_wait) + pending
                pending = []
        assert not pending
        for i in reversed(kill):
            del ins[i]
        removed += len(kill)
    return removed


def _emit_body(nc, tc, chunks, ps_mode, mm_order, wtp, xpp, psp, otp, bp,
               x_in, wt_in, b_in, out_ext, f32, bf16, mybir, ablate="none"):
    wt = wtp.tile([128, 18 * 128], bf16)
    nc.sync.dma_start(out=wt[:], in_=wt_in[:])
    bias = bp.tile([128, 1], f32)
    nc.sync.dma_start(out=bias[:], in_=b_in[:])

    xp = {}

    def load_xpair(j):
        # single [128, 2304] DMA per d-pair; the last f-chunk is 158 wide so
        # every shifted matmul read stays inside the payload (no tail pad)
        t = xpp.tile([128, 2304], bf16, tag="xp")
        nc.sync.dma_start(out=t[:], in_=x_in[:, 2 * j : 2 * j + 2, :])
        xp[j] = t

    load_xpair(0)
    if ablate not in ("no_xload", "mm_only"):
        load_xpair(1)

    ident = mybir.ActivationFunctionType.Identity

    for Q in range(23):
        if Q + 2 <= 23 and ablate not in ("no_xload", "mm_only"):
            load_xpair(Q + 2)
        ot = otp.tile([128, 2208], bf16, tag="ot")
        if Q == 22 and ablate == "none":
            # dummy load: keeps the per-iteration HWDGE-DMA count even (50)
            # so the global round-robin DMAHW sem-lane assignment keeps all
            # loads on even lanes and all stores on odd lanes -- a load's
            # ring-recycle wait then never chains to a store completion
            nc.sync.dma_start(out=ot[:, 2206:2208], in_=x_in[:, 0:2, 0:2])
        # chunks 0-2 double-buffer across Q parity (3*2+2 = 8 PSUM banks):
        # next Q's matmuls never wait on this Q's drains
        pss = [
            psp.tile(
                [128, N], f32,
                tag=f"ps{ci}q{Q % 2}" if ci < 3 else f"ps{ci}",
                name=f"ps{ci}",
            )
            for ci, (f0, N) in enumerate(chunks)
        ]

        def mm(ci, k, s, l, v):
            f0, N = chunks[ci]
            off = f0 + l * 48 + v
            rt = xp[0] if ablate in ("no_xload", "mm_only") else xp[Q + s]
            nc.tensor.matmul(
                pss[ci][:],
                lhsT=wt[:, 128 * k : 128 * (k + 1)],
                rhs=rt[:, off : off + N],
                start=(k == 0),
                stop=(k == 17),
            )

        if mm_order == "weight":
            # stationary-weight reuse: LDW once per (k), 5 matmuls
            for k in range(18):
                s, l, v = k // 9, (k % 9) // 3, k % 3
                for ci in range(len(chunks)):
                    mm(ci, k, s, l, v)
        else:
            for ci in range(len(chunks)):
                for k in range(18):
                    s, l, v = k // 9, (k % 9) // 3, k % 3
                    mm(ci, k, s, l, v)

        # drain PSUM -> ot with bias add, split across DVE and ACT so the
        # banks free up ~2x faster for the next Q's matmuls
        if ablate in ("min_io", "mm_only"):
            # keep only a minimal drain+store of the last (160-col) bank so
            # DCE cannot remove the matmuls (benchmark ablation only)
            f0, N = chunks[-1]
            nc.vector.tensor_scalar_add(
                out=ot[:, f0 : f0 + N], in0=pss[-1][:], scalar1=bias[:]
            )
            nc.scalar.dma_start(
                out=out_ext[:, :, Q, f0 : f0 + N], in_=ot[:, f0 : f0 + N]
            )
            continue
        for ci, (f0, N) in enumerate(chunks):
            if ci < 3:
                nc.vector.tensor_scalar_add(
                    out=ot[:, f0 : f0 + N], in0=pss[ci][:], scalar1=bias[:]
                )
            else:
                nc.scalar.activation(
                    out=ot[:, f0 : f0 + N], in_=pss[ci][:], func=ident,
                    bias=bias[:], scale=1.0,
                )
        nc.scalar.dma_start(out=out_ext[:, :, Q, 0:2206], in_=ot[:, 0:2206])


def _get_program():
    global _PROGRAM
    if _PROGRAM is None:
        _PROGRAM = _build_program()
    return _PROGRAM


def _prepare_host_inputs(x, W, b, G):
    import ml_dtypes

    bf16 = ml_dtypes.bfloat16
    B = x.shape[0]
    # Fold GA product table into the conv kernel:
    # out[b,o,k,d,h,w] = sum G[i,j,k] x[b,c,i,...] W[m,l,v,c,o,j]
    Wt = np.einsum("ijk,mlvcoj->okcimlv", G, W).astype(np.float32)
    Kfold = np.ascontiguousarray(Wt.reshape(64, 64, 3, 3, 3))  # [o', c', m, l, v]

    # 18 stationary matrices: lhsT[k_in = c'*2+dpi, p_out = o'*2+dpo]
    WBIG = np.zeros((128, 18, 128), np.float32)
    L = np.zeros((64, 2, 64, 2), np.float32)  # [c', dpi, o', dpo]
    for s in (0, 1):
        for l in range(3):
            for v in range(3):
                k = s * 9 + l * 3 + v
                L[:] = 0.0
                for dpi in (0, 1):
                    for dpo in (0, 1):
                        m = 2 * s + dpi - dpo
                        if 0 <= m <= 2:
                            L[:, dpi, :, dpo] = Kfold[:, :, m, l, v].T
                WBIG[:, k, :] = L.reshape(128, 128)
    wt_arr = np.ascontiguousarray(WBIG.reshape(128, 18 * 128)).astype(bf16)

    bias_arr = np.repeat(b.reshape(64).astype(np.float32), 2).reshape(128, 1)
    bias_arr = np.ascontiguousarray(bias_arr)

    zrow = np.zeros((64, 1, 2304), np.float32)
    xs = [
        np.ascontiguousarray(
            np.concatenate([x[i].reshape(64, 48, 2304), zrow], axis=1)
        ).astype(bf16)
        for i in range(B)
    ]
    return xs, wt_arr, bias_arr


def kernel(**inputs):
    from concourse.bass_utils import run_bass_kernel_spmd

    x = np.asarray(inputs["x"], np.float32)
    W = np.asarray(inputs["W"], np.float32)
    b = np.asarray(inputs["b"], np.float32)
    G = np.asarray(inputs["G"], np.float32)

    xs, wt_arr, bias_arr = _prepare_host_inputs(x, W, b, G)
    nc = _get_program()
    in_maps = [{"x": xs[i], "wt": wt_arr, "bias": bias_arr} for i in range(8)]
    res = run_bass_kernel_spmd(nc, in_maps, list(range(8)))
    out = np.stack([_unpack_out(res.results[i]["out"]) for i in range(8)], axis=0)
    return out.reshape(8, 8, 8, 46, 46, 46)


def _unpack_out(arr):
    # [o', dp, q, f=h*48+w] -> [o', d=2q+dp, h, w] cropped to w<46, fp32
    a = np.asarray(arr, np.float32).reshape(64, 2, 23, 46, 48)[:, :, :, :, 0:46]
    return np.ascontiguousarray(a.transpose(0, 2, 1, 3, 4)).reshape(64, 46, 46, 46)
